# revision 11
# baseline (speedup 1.0000x reference)
"""Cost-volume kernel for Trainium2 (8 NeuronCores, batch-parallel).

Problem: cost[b, o=(dy,dx), h, w] = PReLU(mean_c(c1[b,c,h,w] *
         pad(warped)[b,c,h+dy,w+dx]), alpha), 81 offsets (9x9), zero pad 4.

Strategy per core (one batch element per NeuronCore):
  - Image tiled 16x8 pixels (th x tw), M=128 pixel tile, b-major partition
    order (m = b8*16 + a).
  - TensorE computes a "gram" tile against the 24x16 warped halo:
    PSUM[m, n] = sum_c c1[c, p_m] * wpad[c, halo_n]  (K=96+96 chunks,
    N=384, bf16 inputs, fp32 accumulate).
  - The 81 cost entries of pixel (a, b8) live at n = (a+dy)*16 + (b8+dx),
    a sheared per-partition window that no SBUF AP can express (partition
    steps cannot carry byte remainders), so the device writes the
    partition-uniform superset window [a*16, a*16+144) for each row-group
    a (partitions {a+16*b8}), and the host finishes with a cheap numpy
    diagonal gather + PReLU + 1/192 scale.

Scheduling (v5):
  - wpad lives in one SBUF tile per channel chunk (rows 4..132 loaded in
    24-32-row chunks on the sync ring, paced by the band loop; the 4 pad
    rows top/bottom are memset on device). Range-granular tile deps let
    band 0 start after the first 0.8MB.
  - the scalar ring carries c1 with prefetch depth 4.
  - per-band staging (3 buffers) and per-band output waves: each band's
    16 per-a gout DMAs issue as soon as its copies land, spread over the
    gpsimd SWDGE ring (software DGE aggregates the scattered 288B window
    runs into ~4.3KB packets that spread across all 16 DMA engines;
    HWDGE write packets stay 288B and pin to 8 engines) plus the sync and
    scalar HWDGE rings, so the write drain overlaps compute instead of
    tailing it.
"""

import numpy as np

B, C, H, W = 8, 192, 128, 160
R = 4
TH, TW = 16, 8                    # pixel tile
HH, HWW = TH + 2 * R, TW + 2 * R  # halo 24 x 16
NCOL = HH * HWW                   # 384 matmul free dim
BANDS = H // TH                   # 8 row bands
TPB = W // TW                     # 20 tiles per band
WIN = 2 * R * HWW + TW + 2 * R    # 144 per-a superset window
PH, PW = H + 2 * R, W + 2 * R     # padded 136 x 168
K0, K1 = 96, 96                   # contraction chunks

_CACHE = {}


def _build():
    if "nc" in _CACHE:
        return _CACHE["nc"]
    import sys
    if "/opt/trn_rl_repo" not in sys.path:
        sys.path.insert(0, "/opt/trn_rl_repo")
    import concourse.mybir as mybir
    import concourse.tile as tile
    from concourse import bacc
    from concourse.bass import AP

    nc = bacc.Bacc(None, target_bir_lowering=False)
    bf16 = mybir.dt.bfloat16
    f32 = mybir.dt.float32

    # c1 pre-tiled on host: [C, band, t, m], m = b8*16 + a
    c1_d = nc.dram_tensor("c1b", [C, H * W], bf16, kind="ExternalInput")
    wp_d = nc.dram_tensor("wpad", [C, PH * PW], bf16, kind="ExternalInput")
    go_d = nc.dram_tensor("gout", [BANDS * TH, TW * TPB * WIN], bf16,
                          kind="ExternalOutput")

    with tile.TileContext(nc) as tc:
        with (
            tc.tile_pool(name="wp", bufs=1) as wp_pool,
            tc.tile_pool(name="c1", bufs=5) as c1_pool,
            tc.tile_pool(name="st", bufs=3) as st_pool,
            tc.tile_pool(name="ps", bufs=4, space="PSUM") as ps_pool,
        ):
            # one persistent padded-warped tile per channel chunk; rows
            # [4,132) are real data, rows [0,4) and [132,136) are pad.
            wp_sb = {}
            for k, kn in enumerate((K0, K1)):
                t = wp_pool.tile([kn, PH * PW], bf16, tag=f"wp{k}")
                wp_sb[k] = t
                nc.gpsimd.memset(t[:, 0:R * PW], 0.0)
                nc.gpsimd.memset(t[:, (PH - R) * PW:PH * PW], 0.0)

            # row chunks: first one small so band 0 unblocks early
            WCHUNKS = ((4, 24), (24, 56), (56, 88), (88, 120), (120, 132))

            def load_wp_chunk(ci):
                ra, rb = WCHUNKS[ci]
                for k, (ks, kn) in enumerate(((0, K0), (K0, K1))):
                    nc.sync.dma_start(
                        wp_sb[k][:, ra * PW:rb * PW],
                        wp_d[ks:ks + kn, ra * PW:rb * PW])

            def load_c1(band, eng):
                tiles = []
                for k, (ks, kn) in enumerate(((0, K0), (K0, K1))):
                    t = c1_pool.tile([kn, TPB * 128], bf16, tag=f"c1_{k}")
                    eng.dma_start(
                        t[:], c1_d[ks:ks + kn,
                                   band * TPB * 128:(band + 1) * TPB * 128])
                    tiles.append(t)
                return tiles

            load_wp_chunk(0)
            load_wp_chunk(1)
            c1_tiles = {0: load_c1(0, nc.scalar)}
            c1_tiles[1] = load_c1(1, nc.scalar)
            load_wp_chunk(2)
            c1_tiles[2] = load_c1(2, nc.scalar)
            c1_tiles[3] = load_c1(3, nc.scalar)

            # PE warm-up burst during the initial DMA window keeps the HAM
            # clock up before the real stream starts.
            warm = c1_pool.tile([128, 512], bf16, tag="warm")
            nc.gpsimd.memset(warm[:], 0.0)
            for _ in range(10):
                ps_w = ps_pool.tile([128, 1024], f32, tag="ps")
                nc.tensor.matmul(ps_w[:, 0:512], warm[:, 0:128],
                                 warm[:, 0:512], start=True, stop=True)

            for band in range(BANDS):
                staged = st_pool.tile([128, TPB * NCOL], bf16, tag="staged")
                sap0 = staged[:]
                srow = sap0.ap[0][0]

                r0 = band * TH
                c1_sb = c1_tiles.pop(band)
                # prefetch four bands ahead; pace wp chunks 3,4
                if band + 4 < BANDS:
                    c1_tiles[band + 4] = load_c1(band + 4, nc.scalar)
                if band == 0:
                    load_wp_chunk(3)
                elif band == 1:
                    load_wp_chunk(4)

                for tp in range(TPB // 2):
                    ps = ps_pool.tile([128, 1024], f32, tag="ps")
                    for hf in range(2):
                        t_i = 2 * tp + hf
                        c0 = t_i * TW
                        for k, kn in enumerate((K0, K1)):
                            a1 = c1_sb[k][:]
                            lhsT = AP(a1.tensor,
                                      a1.offset + t_i * 128,
                                      [[a1.ap[0][0], kn], [1, 128]])
                            a2 = wp_sb[k][:]
                            rhs = AP(a2.tensor,
                                     a2.offset + r0 * PW + c0,
                                     [[a2.ap[0][0], kn],
                                      [PW, HH], [1, HWW]])
                            nc.tensor.matmul(
                                ps[:, hf * 512:hf * 512 + NCOL],
                                lhsT, rhs,
                                start=(k == 0), stop=(k == 1))
                    # one copy moves both tiles' grams; DVE/ACT split
                    pap = ps[:]
                    src2 = AP(pap.tensor, pap.offset,
                              [[pap.ap[0][0], 128], [512, 2], [1, NCOL]])
                    d0 = 2 * tp * NCOL
                    dst2 = staged[:, d0:d0 + 2 * NCOL]
                    if tp % 5 < 3:
                        nc.vector.tensor_copy(dst2, src2)
                    else:
                        nc.scalar.copy(dst2, src2)

                # 16 per-a out-DMAs per band, spread over three rings
                gap = go_d[:]
                rings = (nc.gpsimd, nc.sync, nc.scalar)
                for a in range(TH):
                    src = AP(sap0.tensor,
                             sap0.offset + a * srow + a * HWW,
                             [[TH * srow, TW], [NCOL, TPB], [1, WIN]])
                    dst = AP(gap.tensor,
                             gap.offset + (band * TH + a)
                             * (TW * TPB * WIN),
                             [[TPB * WIN, TW], [WIN, TPB], [1, WIN]])
                    rings[a % 3].dma_start(dst, src)

    nc.finalize()
    _CACHE["nc"] = nc
    return nc


def kernel(c1, warped, alpha):
    import sys
    if "/opt/trn_rl_repo" not in sys.path:
        sys.path.insert(0, "/opt/trn_rl_repo")
    import ml_dtypes
    from concourse.bass_utils import run_bass_kernel_spmd

    nc = _build()
    bf = ml_dtypes.bfloat16

    in_maps = []
    for b in range(B):
        wpad = np.zeros((C, PH, PW), np.float32)
        wpad[:, R:R + H, R:R + W] = warped[b]
        # tile c1: [C, band, a, t, b8] -> [C, band, t, b8, a]; m = b8*16 + a
        c1t = np.asarray(c1[b]).reshape(C, BANDS, TH, TPB, TW)
        c1t = np.ascontiguousarray(c1t.transpose(0, 1, 3, 4, 2))
        in_maps.append({
            "c1b": c1t.reshape(C, H * W).astype(bf),
            "wpad": wpad.reshape(C, PH * PW).astype(bf),
        })

    import os
    trace = bool(int(os.environ.get("COSTVOL_TRACE", "0")))
    res = run_bass_kernel_spmd(nc, in_maps, core_ids=list(range(B)),
                               trace=trace)
    if trace:
        _CACHE["last_exec_time_ns"] = res.exec_time_ns

    # host-side: diagonal gather + mean + PReLU
    a_val = float(np.asarray(alpha).reshape(-1)[0])
    dy, dx = np.meshgrid(np.arange(9), np.arange(9), indexing="ij")
    oidx = (dy * HWW + dx).reshape(-1)                      # [81]
    jidx = np.arange(TW)[:, None] + oidx[None, :]           # [b8, 81]

    out = np.empty((B, 81, H, W), np.float32)
    for b in range(B):
        g = np.asarray(res.results[b]["gout"]).astype(np.float32)
        # [band*16+a, b8, t, j]
        g = g.reshape(BANDS, TH, TW, TPB, WIN)
        got = np.take_along_axis(
            g, jidx[None, None, :, None, :], axis=4)
        # [band, a, b8, t, 81] -> [81, band, a, t, b8] -> [81, h, w]
        cost = got.transpose(4, 0, 1, 3, 2).reshape(81, H, W) * (1.0 / C)
        out[b] = np.where(cost >= 0, cost, a_val * cost)
    return out


# revision 12
# speedup vs baseline: 1.0653x; 1.0653x over previous
"""Cost-volume kernel for Trainium2 (8 NeuronCores, batch-parallel).

Problem: cost[b, o=(dy,dx), h, w] = PReLU(mean_c(c1[b,c,h,w] *
         pad(warped)[b,c,h+dy,w+dx]), alpha), 81 offsets (9x9), zero pad 4.

Strategy per core (one batch element per NeuronCore):
  - Image tiled 16x8 pixels (th x tw), M=128 pixel tile, b-major partition
    order (m = b8*16 + a).
  - TensorE computes a "gram" tile against the 24x16 warped halo:
    PSUM[m, n] = sum_c c1[c, p_m] * wpad[c, halo_n]  (K=96+96 chunks,
    N=384, bf16 inputs, fp32 accumulate).
  - The 81 cost entries of pixel (a, b8) live at n = (a+dy)*16 + (b8+dx),
    a sheared per-partition window that no SBUF AP can express (partition
    steps cannot carry byte remainders), so the device writes the
    partition-uniform superset window [a*16, a*16+144) for each row-group
    a (partitions {a+16*b8}), and the host finishes with a cheap numpy
    diagonal gather + PReLU + 1/192 scale.

Scheduling (v6), driven by the measured DMA-engine model (16 engines;
reads ~15.5-16.5 B/ns each, scattered 288B-run writes ~11.4 B/ns; HWDGE
write packets pin to engines 64-71 while gpsimd-SWDGE write packets are
aggregated ~4.3KB and spread over all 16):
  - wpad in one SBUF tile per channel chunk, rows 4..132 loaded in
    24-32-row chunks on the sync ring (~3.5KB descriptors), paced by the
    band loop; pad rows memset on device. Range-granular tile deps let
    band 0 start after ~0.8MB.
  - the scalar ring carries only c1 (prefetch depth 3).
  - gout waves: groups 0-2 entirely on the gpsimd SWDGE ring (16 DMAs x
    ~1.1us issue per ~22us group cadence, writes spread 16-wide); the
    last group issues per-band across all three rings so the drain tail
    stays short.
"""

import numpy as np

B, C, H, W = 8, 192, 128, 160
R = 4
TH, TW = 16, 8                    # pixel tile
HH, HWW = TH + 2 * R, TW + 2 * R  # halo 24 x 16
NCOL = HH * HWW                   # 384 matmul free dim
BANDS = H // TH                   # 8 row bands
TPB = W // TW                     # 20 tiles per band
WIN = 2 * R * HWW + TW + 2 * R    # 144 per-a superset window
PH, PW = H + 2 * R, W + 2 * R     # padded 136 x 168
K0, K1 = 96, 96                   # contraction chunks
GB = 2                            # bands per staged group
NGRP = BANDS // GB                # 4 staged groups

_CACHE = {}


def _build():
    if "nc" in _CACHE:
        return _CACHE["nc"]
    import sys
    if "/opt/trn_rl_repo" not in sys.path:
        sys.path.insert(0, "/opt/trn_rl_repo")
    import concourse.mybir as mybir
    import concourse.tile as tile
    from concourse import bacc
    from concourse.bass import AP

    nc = bacc.Bacc(None, target_bir_lowering=False)
    bf16 = mybir.dt.bfloat16
    f32 = mybir.dt.float32

    # c1 pre-tiled on host: [C, band, t, m], m = b8*16 + a
    c1_d = nc.dram_tensor("c1b", [C, H * W], bf16, kind="ExternalInput")
    wp_d = nc.dram_tensor("wpad", [C, PH * PW], bf16, kind="ExternalInput")
    go_d = nc.dram_tensor("gout", [NGRP * TH, TW * GB * TPB * WIN], bf16,
                          kind="ExternalOutput")

    with tile.TileContext(nc) as tc:
        with (
            tc.tile_pool(name="wp", bufs=1) as wp_pool,
            tc.tile_pool(name="c1", bufs=4) as c1_pool,
            tc.tile_pool(name="st", bufs=2) as st_pool,
            tc.tile_pool(name="ps", bufs=4, space="PSUM") as ps_pool,
        ):
            # one persistent padded-warped tile per channel chunk; rows
            # [4,132) are real data, rows [0,4) and [132,136) are pad.
            wp_sb = {}
            for k, kn in enumerate((K0, K1)):
                t = wp_pool.tile([kn, PH * PW], bf16, tag=f"wp{k}")
                wp_sb[k] = t
                nc.gpsimd.memset(t[:, 0:R * PW], 0.0)
                nc.gpsimd.memset(t[:, (PH - R) * PW:PH * PW], 0.0)

            # row chunks (first small so band 0 unblocks early) with a
            # last-dim cap that keeps descriptors ~3.4-4KB
            WCHUNKS = ((4, 24, 1680), (24, 56, 1792), (56, 88, 1792),
                       (88, 120, 1792), (120, 132, 2016))

            def load_wp_chunk(ci):
                ra, rb, mld = WCHUNKS[ci]
                for k, (ks, kn) in enumerate(((0, K0), (K0, K1))):
                    nc.sync.dma_start(
                        wp_sb[k][:, ra * PW:rb * PW],
                        wp_d[ks:ks + kn, ra * PW:rb * PW],
                        max_dma_last_dim=mld)

            def load_c1(band, eng):
                tiles = []
                for k, (ks, kn) in enumerate(((0, K0), (K0, K1))):
                    t = c1_pool.tile([kn, TPB * 128], bf16, tag=f"c1_{k}")
                    eng.dma_start(
                        t[:], c1_d[ks:ks + kn,
                                   band * TPB * 128:(band + 1) * TPB * 128])
                    tiles.append(t)
                return tiles

            load_wp_chunk(0)
            load_wp_chunk(1)
            c1_tiles = {0: load_c1(0, nc.scalar)}
            c1_tiles[1] = load_c1(1, nc.scalar)
            load_wp_chunk(2)
            c1_tiles[2] = load_c1(2, nc.scalar)

            # PE warm-up burst during the initial DMA window keeps the HAM
            # clock up before the real stream starts.
            warm = c1_pool.tile([128, 512], bf16, tag="warm")
            nc.gpsimd.memset(warm[:], 0.0)
            for _ in range(10):
                ps_w = ps_pool.tile([128, 1024], f32, tag="ps")
                nc.tensor.matmul(ps_w[:, 0:512], warm[:, 0:128],
                                 warm[:, 0:512], start=True, stop=True)

            for grp in range(NGRP):
                staged = st_pool.tile([128, GB * TPB * NCOL], bf16,
                                      tag="staged")
                sap0 = staged[:]
                srow = sap0.ap[0][0]

                for bb in range(GB):
                    band = grp * GB + bb
                    r0 = band * TH
                    c1_sb = c1_tiles.pop(band)
                    # prefetch three bands ahead; pace wp chunks 3,4
                    if band + 3 < BANDS:
                        c1_tiles[band + 3] = load_c1(band + 3, nc.scalar)
                    if band == 0:
                        load_wp_chunk(3)
                    elif band == 1:
                        load_wp_chunk(4)

                    for tp in range(TPB // 2):
                        ps = ps_pool.tile([128, 1024], f32, tag="ps")
                        for hf in range(2):
                            t_i = 2 * tp + hf
                            c0 = t_i * TW
                            for k, kn in enumerate((K0, K1)):
                                a1 = c1_sb[k][:]
                                lhsT = AP(a1.tensor,
                                          a1.offset + t_i * 128,
                                          [[a1.ap[0][0], kn], [1, 128]])
                                a2 = wp_sb[k][:]
                                rhs = AP(a2.tensor,
                                         a2.offset + r0 * PW + c0,
                                         [[a2.ap[0][0], kn],
                                          [PW, HH], [1, HWW]])
                                nc.tensor.matmul(
                                    ps[:, hf * 512:hf * 512 + NCOL],
                                    lhsT, rhs,
                                    start=(k == 0), stop=(k == 1))
                        # one copy moves both tiles' grams; DVE/ACT split
                        pap = ps[:]
                        src2 = AP(pap.tensor, pap.offset,
                                  [[pap.ap[0][0], 128], [512, 2],
                                   [1, NCOL]])
                        d0 = (bb * TPB + 2 * tp) * NCOL
                        dst2 = staged[:, d0:d0 + 2 * NCOL]
                        if tp % 5 < 3:
                            nc.vector.tensor_copy(dst2, src2)
                        else:
                            nc.scalar.copy(dst2, src2)

                # gout waves: groups 0-2 all on the gpsimd SWDGE ring;
                # last group per-band across three rings for a short tail.
                gap = go_d[:]
                if grp == NGRP - 1:
                    rings = (nc.gpsimd, nc.sync, nc.scalar, nc.gpsimd)
                    for bb in range(GB):
                        for a in range(TH):
                            src = AP(sap0.tensor,
                                     sap0.offset + a * srow + a * HWW
                                     + bb * TPB * NCOL,
                                     [[TH * srow, TW], [NCOL, TPB],
                                      [1, WIN]])
                            dst = AP(gap.tensor,
                                     gap.offset + (grp * TH + a)
                                     * (TW * GB * TPB * WIN)
                                     + bb * TPB * WIN,
                                     [[GB * TPB * WIN, TW], [WIN, TPB],
                                      [1, WIN]])
                            rings[a % 4].dma_start(dst, src)
                else:
                    for a in range(TH):
                        src = AP(sap0.tensor,
                                 sap0.offset + a * srow + a * HWW,
                                 [[TH * srow, TW], [NCOL, GB * TPB],
                                  [1, WIN]])
                        dst = AP(gap.tensor,
                                 gap.offset + (grp * TH + a)
                                 * (TW * GB * TPB * WIN),
                                 [[GB * TPB * WIN, TW], [WIN, GB * TPB],
                                  [1, WIN]])
                        nc.gpsimd.dma_start(dst, src)

    nc.finalize()
    _CACHE["nc"] = nc
    return nc


def kernel(c1, warped, alpha):
    import sys
    if "/opt/trn_rl_repo" not in sys.path:
        sys.path.insert(0, "/opt/trn_rl_repo")
    import ml_dtypes
    from concourse.bass_utils import run_bass_kernel_spmd

    nc = _build()
    bf = ml_dtypes.bfloat16

    in_maps = []
    for b in range(B):
        wpad = np.zeros((C, PH, PW), np.float32)
        wpad[:, R:R + H, R:R + W] = warped[b]
        # tile c1: [C, band, a, t, b8] -> [C, band, t, b8, a]; m = b8*16 + a
        c1t = np.asarray(c1[b]).reshape(C, BANDS, TH, TPB, TW)
        c1t = np.ascontiguousarray(c1t.transpose(0, 1, 3, 4, 2))
        in_maps.append({
            "c1b": c1t.reshape(C, H * W).astype(bf),
            "wpad": wpad.reshape(C, PH * PW).astype(bf),
        })

    import os
    trace = bool(int(os.environ.get("COSTVOL_TRACE", "0")))
    res = run_bass_kernel_spmd(nc, in_maps, core_ids=list(range(B)),
                               trace=trace)
    if trace:
        _CACHE["last_exec_time_ns"] = res.exec_time_ns

    # host-side: diagonal gather + mean + PReLU
    a_val = float(np.asarray(alpha).reshape(-1)[0])
    dy, dx = np.meshgrid(np.arange(9), np.arange(9), indexing="ij")
    oidx = (dy * HWW + dx).reshape(-1)                      # [81]
    jidx = np.arange(TW)[:, None] + oidx[None, :]           # [b8, 81]

    out = np.empty((B, 81, H, W), np.float32)
    for b in range(B):
        g = np.asarray(res.results[b]["gout"]).astype(np.float32)
        # [grp*16+a, b8, band2, t, j]
        g = g.reshape(NGRP, TH, TW, GB, TPB, WIN)
        got = np.take_along_axis(
            g, jidx[None, None, :, None, None, :], axis=5)
        # -> [81, grp, band2, a, t, b8] -> [81, h, w]
        cost = got.transpose(5, 0, 3, 1, 4, 2).reshape(81, H, W) * (1.0 / C)
        out[b] = np.where(cost >= 0, cost, a_val * cost)
    return out
